# revision 92
# baseline (speedup 1.0000x reference)
"""Trainium2 Bass kernel: per-cluster PCA geometry features (segment reduce).

Problem: data [4194304, 6] f32, clusts [32768, 128] int — per cluster of 128
voxels compute: center (mean of xyz), normalized covariance B = A/lmax,
principal axis v0 scaled by dirwt = 1 - lmid/lmax with a sign fix, size.

Strategy (v4): shard the 32768 clusters across 8 NeuronCores (4096 each).
Host pre-gathers each cluster's voxel coords (pure permutation), casts to
bf16, and ships TWO layouts per core:
  voxel-major  xt/yt/zt [128 vox, 4096 clusters] — phase-1 moment sums run
    on the PE (column sums via ones-rhs matmuls, nearly free).
  cluster-major xc/yc/zc [128 part, 128 vox, 16 seg] per half — phase-2
    element ops. Segment-INNERMOST layout keeps every DVE operand's last AP
    dim stride-1 so bf16 ops hit the 2x DVE mode, including per-cluster
    broadcasts (stride-0 on the middle/voxel dim only).
Cluster c = g*128 + q maps to (partition q, segment g), matching the PE
column-sum output layout, so moments land directly where the eigensolve
([128, 32] fp32 small-tile analytic 3x3 solve, trig method) wants them.
Input DMAs are split across the SP/ACT/Pool issue queues so transfers
overlap; work is split across DVE/ACT/Pool by measured cost-model rates
(Pool subtract is cheaper than mult); ACT table switches (sqrt<->trig
sets) are batched; feature values are written straight into the output
tile; tails and output DMA run per half.
"""
import numpy as np
from contextlib import ExitStack

import concourse.bass as bass
import concourse.bacc as bacc
import concourse.tile as tile
from concourse import mybir
from concourse.bass_utils import run_bass_kernel_spmd

N_CLUSTS = 32768
CLUST_SIZE = 128
N_CORES = 8
C_LOC = N_CLUSTS // N_CORES   # 4096 clusters per core
P = 128                       # SBUF partitions
NSEG = C_LOC // P             # 32 clusters (segments) per partition
V = CLUST_SIZE                # 128 voxels per cluster
NH = 2                        # halves for pipelining
GH = NSEG // NH               # 16 segments per half
CH = C_LOC // NH              # 2048 clusters per half

F32 = mybir.dt.float32
BF16 = mybir.dt.bfloat16
U8 = mybir.dt.uint8
AF = mybir.ActivationFunctionType
OP = mybir.AluOpType
AX = mybir.AxisListType

PI_2 = 1.5707963267948966
PI_6 = 0.5235987755982988

_CACHED = {}


def build_nc():
    nc = bacc.Bacc()
    xt_d = nc.dram_tensor("xt", [V, C_LOC], BF16, kind="ExternalInput").ap()
    yt_d = nc.dram_tensor("yt", [V, C_LOC], BF16, kind="ExternalInput").ap()
    zt_d = nc.dram_tensor("zt", [V, C_LOC], BF16, kind="ExternalInput").ap()
    xc_d = nc.dram_tensor("xc", [NH, P, V, GH], BF16, kind="ExternalInput").ap()
    yc_d = nc.dram_tensor("yc", [NH, P, V, GH], BF16, kind="ExternalInput").ap()
    zc_d = nc.dram_tensor("zc", [NH, P, V, GH], BF16, kind="ExternalInput").ap()
    feats_d = nc.dram_tensor("feats", [NSEG, P, 16], F32, kind="ExternalOutput").ap()

    with tile.TileContext(nc) as tc, ExitStack() as ctx:
        pool = ctx.enter_context(tc.tile_pool(name="main", bufs=1))
        sp = ctx.enter_context(tc.tile_pool(name="p1s", bufs=6))
        p2p = ctx.enter_context(tc.tile_pool(name="p2s", bufs=1))
        pp = ctx.enter_context(tc.tile_pool(name="psum", bufs=2, space="PSUM"))

        D = nc.vector   # DVE
        A = nc.scalar   # Activation
        G = nc.gpsimd   # Pool

        ones = pool.tile([P, 1], BF16, tag="ones")
        G.memset(ones[:], 1.0)
        bias_pi2 = pool.tile([P, 1], F32, tag="bias_pi2")
        bias_pi6 = pool.tile([P, 1], F32, tag="bias_pi6")
        G.memset(bias_pi2[:], PI_2)
        G.memset(bias_pi6[:], PI_6)
        bias_eps = pool.tile([P, 1], F32, tag="bias_eps")
        G.memset(bias_eps[:], 1e-30)
        bias_half = pool.tile([P, 1], F32, tag="bias_half")
        G.memset(bias_half[:], 0.5)
        bias_one = pool.tile([P, 1], F32, tag="bias_one")
        G.memset(bias_one[:], 1.0)
        dum = pool.tile([P, 1], F32, tag="dum")
        A.activation(bias_pi6[:], bias_pi2[:], AF.Sqrt)
        G.memset(bias_pi6[:], PI_6)

        # ---- input DMAs, split across issue queues so transfers overlap ----
        vm = {}   # (coord, half) -> [P, CH] bf16 voxel-major
        cm = {}   # (coord, half) -> [P, V, GH] bf16 cluster-major seg-inner
        for h in range(NH):
            veng = nc.sync if h == 0 else nc.gpsimd
            for k, (name, d) in enumerate(
                    (("x", xt_d), ("y", yt_d), ("z", zt_d))):
                t = pool.tile([P, CH], BF16, tag=f"vm_{name}{h}", name=f"vm_{name}{h}")
                veng.dma_start(t[:], d[:, h * CH:(h + 1) * CH])
                vm[(k, h)] = t
        for h in range(NH):
            eng = nc.sync
            for k, (name, d) in enumerate(
                    (("x", xc_d), ("y", yc_d), ("z", zc_d))):
                t = pool.tile([P, V, GH], BF16, tag=f"cm_{name}{h}", name=f"cm_{name}{h}")
                eng.dma_start(t[:], d[h])
                cm[(k, h)] = t

        # ---- shared tiles / helpers ----
        ps = [pp.tile([P, 9 * GH], F32, tag=f"ps{h}", name=f"ps{h}")
              for h in range(NH)]
        moments = pool.tile([P, 9, NSEG], F32, tag="moments")
        Sx = moments[:, 0]; Sy = moments[:, 1]; Sz = moments[:, 2]
        Mxx = moments[:, 3]; Myy = moments[:, 4]; Mzz = moments[:, 5]
        Mxy = moments[:, 6]; Mxz = moments[:, 7]; Myz = moments[:, 8]

        feats = pool.tile([P, NSEG, 16], F32, tag="feats")

        def small(name, dt=F32):
            return pool.tile([P, NSEG], dt, tag=f"s_{name}", name=name)

        def ap(x):
            return x[:] if hasattr(x, "tag") else x

        def tt(eng, out, a, b, op):
            eng.tensor_tensor(ap(out), ap(a), ap(b), op)

        def ts(eng, out, in0, s1, s2=None, op0=OP.mult, op1=None):
            kw = dict(out=ap(out), in0=ap(in0), scalar1=s1, scalar2=s2, op0=op0)
            if op1 is not None:
                kw["op1"] = op1
            eng.tensor_scalar(**kw)

        def stt(eng, out, in0, s, in1, op0, op1):
            eng.scalar_tensor_tensor(out=ap(out), in0=ap(in0), scalar=s,
                                     in1=ap(in1), op0=op0, op1=op1)

        inv_s = 1.0 / V
        cxb = small("cxb", BF16); cyb = small("cyb", BF16); czb = small("czb", BF16)

        # ---- phase 1: moments via ACT/DVE/Pool products + PE column sums ----
        def colsum(h, plane, k):
            # column sums of [128, CH] plane: group g -> psum[:, k*GH+g]
            for g in range(GH):
                nc.tensor.matmul(
                    out=ps[h][:, k * GH + g: k * GH + g + 1],
                    lhsT=plane[:, g * P:(g + 1) * P],
                    rhs=ones[:, 0:1], start=True, stop=True)

        def p1_steps(h):
            x, y, z = vm[(0, h)], vm[(1, h)], vm[(2, h)]
            sqx = sp.tile([P, CH], BF16, tag="p1s", name=f"sqx{h}")
            sqy = sp.tile([P, CH], BF16, tag="p1s", name=f"sqy{h}")
            sqz = sp.tile([P, CH], BF16, tag="p1s", name=f"sqz{h}")
            cxy = sp.tile([P, CH], BF16, tag="p1s", name=f"cxy{h}")
            cxz = sp.tile([P, CH], BF16, tag="p1s", name=f"cxz{h}")
            cyz = sp.tile([P, CH], BF16, tag="p1s", name=f"cyz{h}")
            def st1():
                colsum(h, x, 0)
                D.tensor_tensor(sqx[:], x[:], x[:], OP.mult)
                colsum(h, sqx, 3)
            yield st1
            def st2():
                colsum(h, y, 1)
                A.activation(sqy[:], y[:], AF.Square)
                colsum(h, sqy, 4)
                D.tensor_tensor(cxy[:], x[:], y[:], OP.mult)
                colsum(h, cxy, 6)
            yield st2
            def st3():
                colsum(h, z, 2)
                A.activation(sqz[:], z[:], AF.Square)
                colsum(h, sqz, 5)
                D.tensor_tensor(cxz[:], x[:], z[:], OP.mult)
                colsum(h, cxz, 7)
                G.tensor_tensor(cyz[:], y[:], z[:], OP.mult)
                colsum(h, cyz, 8)
            yield st3
            def st3b():
                # raw sums only -> centers unblock before product colsums
                hs = slice(h * GH, (h + 1) * GH)
                D.tensor_copy(
                    moments[:, 0:3, hs],
                    ps[h][:, 0:3 * GH].rearrange("p (k g) -> p k g", k=3))
                ts(D, feats[:, hs, 0], Sx[:, hs], inv_s)
                ts(D, feats[:, hs, 1], Sy[:, hs], inv_s)
                ts(D, feats[:, hs, 2], Sz[:, hs], inv_s)
                D.tensor_copy(cxb[:, hs], feats[:, hs, 0])
                D.tensor_copy(cyb[:, hs], feats[:, hs, 1])
                D.tensor_copy(czb[:, hs], feats[:, hs, 2])
            yield st3b
            def st4():
                D.tensor_copy(
                    moments[:, 3:9, h * GH:(h + 1) * GH],
                    ps[h][:, 3 * GH:].rearrange("p (k g) -> p k g", k=6))
            yield st4

        def zipper(gens):
            done = [False] * len(gens)
            while not all(done):
                for i, g in enumerate(gens):
                    if done[i]:
                        continue
                    try:
                        next(g)()
                    except StopIteration:
                        done[i] = True

        zipper([p1_steps(0), p1_steps(1)])

        # ---- phase 2a in 4 zippered quarters (needs only the centers) ----
        NQ = 4
        GQ = NSEG // NQ   # 8 segments per quarter
        OFFS = [0, 8, 16, 26]
        SZS = [8, 8, 10, 6]

        def bcq(t, qq):
            o, s = OFFS[qq], SZS[qq]
            return t[:, None, o:o + s].broadcast_to([P, V, s])

        Xc = [None] * NQ; Yc = [None] * NQ; Zc = [None] * NQ; Ssum = [None] * NQ

        def cmq(k, qq):
            o, s = OFFS[qq], SZS[qq]
            h = 0 if o < GH else 1
            r = o - h * GH
            return cm[(k, h)][:, :, r:r + s]

        sxs = [None] * NQ; sys_ = [None] * NQ; szs = [None] * NQ

        def p2a_steps(qq):
            Xc[qq] = p2p.tile([P, V, SZS[qq]], BF16, tag=f"Xc{qq}", name=f"Xc{qq}")
            Yc[qq] = p2p.tile([P, V, SZS[qq]], BF16, tag=f"Yc{qq}", name=f"Yc{qq}")
            Zc[qq] = p2p.tile([P, V, SZS[qq]], BF16, tag=f"Zc{qq}", name=f"Zc{qq}")
            yield lambda: G.tensor_tensor(Xc[qq][:], cmq(0, qq), bcq(cxb, qq), OP.subtract)
            yield lambda: G.tensor_tensor(Yc[qq][:], cmq(1, qq), bcq(cyb, qq), OP.subtract)
            yield lambda: D.tensor_tensor(Zc[qq][:], cmq(2, qq), bcq(czb, qq), OP.subtract)
            sxs[qq] = p2p.tile([P, V, SZS[qq]], BF16, tag=f"sx{qq}", name=f"sx{qq}")
            sys_[qq] = p2p.tile([P, V, SZS[qq]], BF16, tag=f"sy{qq}", name=f"sy{qq}")
            szs[qq] = p2p.tile([P, V, SZS[qq]], BF16, tag=f"sz{qq}", name=f"sz{qq}")
            yield lambda: A.activation(sxs[qq][:], Xc[qq][:], AF.Square)
            yield lambda: G.tensor_tensor(sys_[qq][:], Yc[qq][:], Yc[qq][:], OP.mult)
            yield lambda: D.tensor_tensor(szs[qq][:], Zc[qq][:], Zc[qq][:], OP.mult)
            yield lambda: G.tensor_tensor(sxs[qq][:], sxs[qq][:], sys_[qq][:], OP.add)
            Ssum[qq] = p2p.tile([P, V, SZS[qq]], BF16, tag=f"s{qq}", name=f"s{qq}")
            yield lambda: G.tensor_tensor(Ssum[qq][:], sxs[qq][:], szs[qq][:], OP.add)

        zipper([p2a_steps(qq) for qq in range(NQ)])

        # ---- eigen: per-half [128, 16] fp32 analytic 3x3 eigensolve,
        #      stage-grouped so ACT table loads batch (sqrt -> trig -> sqrt)
        axx = small("axx"); ayy = small("ayy"); azz = small("azz")
        axy = small("axy"); axz = small("axz"); ayz = small("ayz")
        t0 = small("t0"); t1 = small("t1"); t2 = small("t2")
        t3 = small("t3"); t4 = small("t4"); t5 = small("t5")
        q = small("q")
        b11 = small("b11"); b22 = small("b22"); b33 = small("b33")
        p2t = small("p2t"); p_ = small("p_"); invp = small("invp")
        r = small("r"); sa = small("sa"); sb = small("sb")
        at4 = small("at4"); cmax = small("cmax"); smin = small("smin")
        w3 = small("w3"); w2 = small("w2")
        invw3 = small("invw3"); dirwt = small("dirwt")
        d1 = small("d1"); d2 = small("d2")
        u1 = small("u1"); u2 = small("u2"); u3 = small("u3")
        k1 = small("k1"); k2 = small("k2")
        nu = small("nu"); nk = small("nk"); nu1 = small("nu1")
        m = small("m", U8)
        e1 = small("e1"); e2 = small("e2"); e3 = small("e3"); ne = small("ne")
        rsn = small("rsn"); invn = small("invn")
        v0xb = small("v0xb", BF16); v0yb = small("v0yb", BF16)
        v0zb = small("v0zb", BF16)

        def eA(h, span=1):
            s_ = slice(h * GH, (h + span) * GH)
            def S(t):
                return t[:, s_]
            A.activation(S(t0), Sx[:, s_], AF.Square)
            stt(D, S(axx), S(t0), -inv_s, Mxx[:, s_], OP.mult, OP.add)
            A.activation(S(t1), Sy[:, s_], AF.Square)
            stt(D, S(ayy), S(t1), -inv_s, Myy[:, s_], OP.mult, OP.add)
            A.activation(S(t2), Sz[:, s_], AF.Square)
            stt(D, S(azz), S(t2), -inv_s, Mzz[:, s_], OP.mult, OP.add)
            tt(D, S(t3), Sx[:, s_], Sy[:, s_], OP.mult)
            stt(D, S(axy), S(t3), -inv_s, Mxy[:, s_], OP.mult, OP.add)
            tt(D, S(t4), Sx[:, s_], Sz[:, s_], OP.mult)
            stt(D, S(axz), S(t4), -inv_s, Mxz[:, s_], OP.mult, OP.add)
            tt(D, S(t5), Sy[:, s_], Sz[:, s_], OP.mult)
            stt(D, S(ayz), S(t5), -inv_s, Myz[:, s_], OP.mult, OP.add)
            tt(D, S(t0), S(axx), S(ayy), OP.add)
            tt(D, S(t0), S(t0), S(azz), OP.add)
            ts(D, S(q), S(t0), 1.0 / 3.0)
            tt(D, S(b11), S(axx), S(q), OP.subtract)
            tt(D, S(b22), S(ayy), S(q), OP.subtract)
            tt(D, S(b33), S(azz), S(q), OP.subtract)
            A.activation(S(t0), S(b11), AF.Square)
            A.activation(S(t1), S(b22), AF.Square)
            A.activation(S(t2), S(b33), AF.Square)
            A.activation(S(t3), S(axy), AF.Square)
            A.activation(S(t4), S(axz), AF.Square)
            A.activation(S(t5), S(ayz), AF.Square)
            tt(D, S(t0), S(t0), S(t1), OP.add)
            tt(D, S(t0), S(t0), S(t2), OP.add)
            tt(D, S(t3), S(t3), S(t4), OP.add)
            tt(D, S(t3), S(t3), S(t5), OP.add)
            stt(D, S(p2t), S(t3), 2.0, S(t0), OP.mult, OP.add)
            A.activation(S(p_), S(p2t), AF.Sqrt, scale=1.0 / 6.0)
            D.reciprocal(S(invp), S(p_))

        def eBsqrt(h, span=1):
            s_ = slice(h * GH, (h + span) * GH)
            def S(t):
                return t[:, s_]
            tt(D, S(t0), S(b22), S(b33), OP.mult)
            tt(D, S(t1), S(ayz), S(ayz), OP.mult)
            tt(D, S(t0), S(t0), S(t1), OP.subtract)
            tt(D, S(t0), S(t0), S(b11), OP.mult)
            tt(D, S(t2), S(axy), S(b33), OP.mult)
            tt(D, S(t3), S(ayz), S(axz), OP.mult)
            tt(D, S(t2), S(t2), S(t3), OP.subtract)
            tt(D, S(t2), S(t2), S(axy), OP.mult)
            tt(D, S(t4), S(axy), S(ayz), OP.mult)
            tt(D, S(t5), S(b22), S(axz), OP.mult)
            tt(D, S(t4), S(t4), S(t5), OP.subtract)
            tt(D, S(t4), S(t4), S(axz), OP.mult)
            tt(D, S(t0), S(t0), S(t2), OP.subtract)
            tt(D, S(t0), S(t0), S(t4), OP.add)
            tt(D, S(t1), S(invp), S(invp), OP.mult)
            tt(D, S(t1), S(t1), S(invp), OP.mult)
            tt(D, S(t0), S(t0), S(t1), OP.mult)
            ts(D, S(r), S(t0), 0.5, 1.0, OP.mult, OP.min)
            ts(D, S(r), S(r), -1.0, None, OP.max)
            A.activation(S(sa), S(r), AF.Sqrt, bias=bias_half[:, 0:1],
                         scale=-0.5)
            A.activation(S(sb), S(r), AF.Sqrt, bias=bias_half[:, 0:1],
                         scale=0.5)
            A.activation(dum[:], bias_pi2[:, 0:1], AF.Arctan)
            ts(D, S(sb), S(sb), 1.0, None, OP.add)
            tt(D, S(t3), S(sa), S(sb), OP.divide)

        def eBtrig(h, span=1):
            s_ = slice(h * GH, (h + span) * GH)
            def S(t):
                return t[:, s_]
            A.activation(S(at4), S(t3), AF.Arctan)
            A.activation(S(cmax), S(at4), AF.Sin, bias=bias_pi2[:, 0:1],
                         scale=-4.0 / 3.0)
            A.activation(S(smin), S(at4), AF.Sin, bias=bias_pi6[:, 0:1],
                         scale=4.0 / 3.0)
            A.activation(dum[:], bias_pi2[:, 0:1], AF.Sqrt)

        def eC(h, span=1):
            s_ = slice(h * GH, (h + span) * GH)
            def S(t):
                return t[:, s_]
            tt(D, S(t0), S(p_), S(cmax), OP.mult)
            stt(D, S(w3), S(t0), 2.0, S(q), OP.mult, OP.add)
            tt(D, S(t1), S(p_), S(smin), OP.mult)
            stt(D, S(t1), S(t1), -2.0, S(q), OP.mult, OP.add)      # w1
            stt(D, S(t2), S(q), 3.0, S(w3), OP.mult, OP.subtract)  # 3q - w3
            tt(D, S(w2), S(t2), S(t1), OP.subtract)
            D.reciprocal(S(invw3), S(w3))
            tt(D, S(t0), S(w2), S(invw3), OP.mult)
            ts(D, S(dirwt), S(t0), -1.0, 1.0, OP.mult, OP.add)
            fs = feats[:, s_, :]
            tt(D, fs[:, :, 3], S(axx), S(invw3), OP.mult)
            tt(D, fs[:, :, 4], S(axy), S(invw3), OP.mult)
            A.copy(fs[:, :, 6], fs[:, :, 4])
            tt(D, fs[:, :, 5], S(axz), S(invw3), OP.mult)
            A.copy(fs[:, :, 9], fs[:, :, 5])
            tt(D, fs[:, :, 7], S(ayy), S(invw3), OP.mult)
            tt(D, fs[:, :, 8], S(ayz), S(invw3), OP.mult)
            A.copy(fs[:, :, 10], fs[:, :, 8])
            tt(D, fs[:, :, 11], S(azz), S(invw3), OP.mult)
            tt(D, S(d1), S(axx), S(w3), OP.subtract)
            tt(D, S(d2), S(ayy), S(w3), OP.subtract)
            tt(D, S(t0), S(axy), S(ayz), OP.mult)
            tt(D, S(t1), S(d2), S(axz), OP.mult)
            tt(D, S(u1), S(t0), S(t1), OP.subtract)
            tt(D, S(t2), S(axy), S(axz), OP.mult)
            tt(D, S(t3), S(d1), S(ayz), OP.mult)
            tt(D, S(u2), S(t2), S(t3), OP.subtract)
            tt(D, S(t4), S(d1), S(d2), OP.mult)
            tt(D, S(t5), S(axy), S(axy), OP.mult)
            tt(D, S(u3), S(t4), S(t5), OP.subtract)
            tt(D, S(nu1), S(u1), S(u1), OP.mult)
            tt(D, S(t0), S(u2), S(u2), OP.mult)
            tt(D, S(t1), S(u3), S(u3), OP.mult)
            tt(D, S(t0), S(t0), S(nu1), OP.add)
            tt(D, S(nu), S(t0), S(t1), OP.add)
            A.activation(S(rsn), S(nu), AF.Sqrt, bias=bias_eps[:, 0:1])
            D.reciprocal(S(invn), S(rsn))
            tt(D, S(v0xb), S(u1), S(invn), OP.mult)
            tt(D, S(v0yb), S(u2), S(invn), OP.mult)
            tt(D, S(v0zb), S(u3), S(invn), OP.mult)

        eA(0, NSEG // GH)
        eBsqrt(0, NSEG // GH)
        eBtrig(0, NSEG // GH)
        eC(0, NSEG // GH)

        # ---- phase 2b: projections, residual norms, sign criterion ----
        sc = small("sc")
        G.memset(feats[:, :, 15], float(V))

        def p2b_steps(qq):
            a1 = p2p.tile([P, V, SZS[qq]], BF16, tag=f"a1{qq}", name=f"a1{qq}")
            a2 = p2p.tile([P, V, SZS[qq]], BF16, tag=f"a2{qq}", name=f"a2{qq}")
            a3 = p2p.tile([P, V, SZS[qq]], BF16, tag=f"a3{qq}", name=f"a3{qq}")
            yield lambda: D.tensor_tensor(a1[:], Xc[qq][:], bcq(v0xb, qq), OP.mult)
            yield lambda: G.tensor_tensor(a2[:], Yc[qq][:], bcq(v0yb, qq), OP.mult)
            yield lambda: G.tensor_tensor(a3[:], Zc[qq][:], bcq(v0zb, qq), OP.mult)
            x0 = p2p.tile([P, V, SZS[qq]], BF16, tag=f"x0{qq}", name=f"x0{qq}")
            yield lambda: D.tensor_tensor(x0[:], a1[:], a2[:], OP.add)
            yield lambda: D.tensor_tensor(x0[:], x0[:], a3[:], OP.add)
            q2 = p2p.tile([P, V, SZS[qq]], BF16, tag=f"q2{qq}", name=f"q2{qq}")
            yield lambda: A.activation(q2[:], x0[:], AF.Square)
            yield lambda: G.tensor_tensor(q2[:], Ssum[qq][:], q2[:], OP.subtract)
            yield lambda: ts(D, q2, q2, 0.0, None, OP.max)
            yield lambda: A.activation(q2[:], q2[:], AF.Sqrt)
            yield lambda: G.tensor_tensor(x0[:], x0[:], q2[:], OP.mult)
            yield lambda: D.tensor_tensor(
                x0[:, 0:V // 2], x0[:, 0:V // 2], x0[:, V // 2:V], OP.add)
            yield lambda: D.tensor_tensor(
                x0[:, 0:V // 4], x0[:, 0:V // 4], x0[:, V // 4:V // 2], OP.add)
            yield lambda: D.tensor_reduce(sc[:, OFFS[qq]:OFFS[qq] + SZS[qq]],
                                          x0[:, 0:V // 4].rearrange("p v g -> p g v"),
                                          axis=AX.X, op=OP.add)
            qs = slice(OFFS[qq], OFFS[qq] + SZS[qq])
            yield lambda: ts(D, t0[:, qs], sc[:, qs], 0.0, -2.0, OP.is_lt, OP.mult)
            yield lambda: ts(D, t0[:, qs], t0[:, qs], 1.0, None, OP.add)
            yield lambda: tt(D, t1[:, qs], t0[:, qs], dirwt[:, qs], OP.mult)
            yield lambda: tt(D, feats[:, qs, 12], v0xb[:, qs], t1[:, qs], OP.mult)
            yield lambda: tt(D, feats[:, qs, 13], v0yb[:, qs], t1[:, qs], OP.mult)
            yield lambda: tt(D, feats[:, qs, 14], v0zb[:, qs], t1[:, qs], OP.mult)
            oeng = nc.scalar if qq < 3 else nc.sync
            yield lambda: oeng.dma_start(
                feats_d[OFFS[qq]:OFFS[qq] + SZS[qq]].rearrange("g q f -> q g f"),
                feats[:, qs, :])

        zipper([p2b_steps(qq) for qq in range(NQ)])

    if not nc.is_finalized():
        nc.finalize()
    return nc


def kernel(data: np.ndarray, clusts: np.ndarray) -> np.ndarray:
    import ml_dtypes
    data = np.asarray(data, dtype=np.float32)
    clusts_np = np.asarray(clusts)
    C, S = clusts_np.shape
    assert (C, S) == (N_CLUSTS, CLUST_SIZE), (C, S)

    vox = data[:, 1:4]
    g3 = vox[clusts_np.reshape(-1).astype(np.int64)].reshape(C, S, 3)
    g3 = g3.astype(ml_dtypes.bfloat16)

    if "nc" not in _CACHED:
        _CACHED["nc"] = build_nc()
    nc = _CACHED["nc"]

    in_maps = []
    for c in range(N_CORES):
        a = g3[c * C_LOC:(c + 1) * C_LOC]          # [4096, 128, 3]
        vmt = np.ascontiguousarray(a.transpose(1, 0, 2))  # [128 vox, 4096, 3]
        # cluster-major seg-inner: [h, q, v, g] with c = (h*GH+g)*128 + q
        b = a.reshape(NH, GH, P, V, 3).transpose(0, 2, 3, 1, 4)
        b = np.ascontiguousarray(b)                # [2, 128, 128, 16, 3]
        in_maps.append({
            "xt": np.ascontiguousarray(vmt[:, :, 0]),
            "yt": np.ascontiguousarray(vmt[:, :, 1]),
            "zt": np.ascontiguousarray(vmt[:, :, 2]),
            "xc": np.ascontiguousarray(b[..., 0]),
            "yc": np.ascontiguousarray(b[..., 1]),
            "zc": np.ascontiguousarray(b[..., 2]),
        })

    res = run_bass_kernel_spmd(nc, in_maps, list(range(N_CORES)))
    out = np.concatenate(
        [res.results[c]["feats"].reshape(C_LOC, 16) for c in range(N_CORES)],
        axis=0)
    return out.astype(np.float32)
